# revision 4
# baseline (speedup 1.0000x reference)
"""DeepSeek-MLA prefill kernel for 8 Trainium2 NeuronCores.

Sharding: tensor-parallel over heads (2 heads/core), zero collectives.
Per core (all bf16 on the PE):
  1. Fold low-rank q/kv projections into effective weights on the PE:
     weff = (w_up_slice @ w_down).T, with both down matrices SBUF-resident.
  2. Single fused apply pass over 512-token chunks: q/k feature-major
     (weff stationary), v token-major (x stationary, no transposes).
     Per-chunk RoPE on DVE + RMS-norm sum-of-squares via one ones-stationary
     matmul per (head, chunk); q's norm scale multiplied into qattn via a
     DRAM-broadcast, k's norm scale kept per-token for the exp scale.
  3. Attention computed TRANSPOSED: S^T[kv, q] = k_j^T q per kv tile j
     (stationary k tile), exp on ScalarE with per-partition scale = k-norm,
     causal masks as elementwise multiplies, PV consumes exp(S^T) directly
     (stationary v tile, token-major), denominators via ones-stationary
     matmuls accumulated in a partition-sliced PSUM bank.
  4. y^T scaled by 1/denom (DRAM broadcast) into bf16 yt; wo per completed
     512-query chunk overlapped with the remaining attention.
Host sums the 8 partial outputs (the all-reduce after wo).
"""

import os
import sys

os.environ.setdefault("JAX_PLATFORMS", "axon,cpu")
if "/opt/trn_rl_repo" not in sys.path:
    sys.path.insert(0, "/opt/trn_rl_repo")

import numpy as np

import concourse.bass as bass
import concourse.tile as tile
from concourse import bacc, mybir
from concourse.bass import ts
from concourse.bass_utils import run_bass_kernel_spmd
from concourse.masks import make_identity

B, T, C = 2, 2048, 2048
H = 16
ROPE_DIM, NOPE_DIM, V_DIM = 64, 64, 128
HEAD_DIM = NOPE_DIM + ROPE_DIM
Q_RANK, KV_RANK = 1536, 512
NCORES = 8
HPC = H // NCORES          # 2 heads per core
NT = B * T                 # 4096 tokens
P = 128
KQ, KK, KC = Q_RANK // P, KV_RANK // P, C // P  # 12, 4, 16
TCH = 512                  # apply-phase token chunk
NCH = NT // TCH            # 8
TQT = T // P               # 16 kv tiles per batch
NQC = T // TCH             # 4 query chunks per batch
EPS = float(np.finfo(np.float32).eps)
EPS128 = EPS * HEAD_DIM

F32 = mybir.dt.float32
BF16 = mybir.dt.bfloat16
EXP = mybir.ActivationFunctionType.Exp
SQRT = mybir.ActivationFunctionType.Sqrt
SQUARE = mybir.ActivationFunctionType.Square
COPY = mybir.ActivationFunctionType.Copy
MULT = mybir.AluOpType.mult

_CACHE = {}
_last_results = None


def _cp(nc, idx, out, in_):
    """Alternate PSUM->SBUF copies between vector and scalar engines."""
    if idx % 2 == 0:
        nc.vector.tensor_copy(out, in_)
    else:
        nc.scalar.activation(out, in_, COPY)


def _fold(nc, tc, tag, down_t, up_t, K, opair, o_off, weff, nonT, identb):
    """weff[:, cc16, o_off:o_off+256] = (up[:, opair] @ down).T for a pair of
    128-wide output groups. down_t [P, K, C], up_t [P, K, *] SBUF bf16."""
    ci = 0
    with tc.tile_pool(name=f"facc{tag}", bufs=1, space="PSUM") as fps:
        pss = {}
        for k in range(K):
            for oi, o in enumerate(opair):
                for cc in range(4):
                    if k == 0:
                        pss[(oi, cc)] = fps.tile([P, 512], F32, tag=f"f{oi}_{cc}",
                                                 name=f"f{tag}{oi}_{cc}")
                    nc.tensor.matmul(pss[(oi, cc)][:], up_t[:, k, ts(o, P)],
                                     down_t[:, k, ts(cc, 512)],
                                     start=(k == 0), stop=(k == K - 1))
        for oi in range(2):
            for cc in range(4):
                _cp(nc, ci, nonT[:, oi, ts(cc, 512)], pss[(oi, cc)][:])
                ci += 1
    with tc.tile_pool(name=f"ftr{tag}", bufs=2, space="PSUM") as ftr:
        for cg in range(8):  # pairs of 128-wide C tiles
            pt = ftr.tile([P, 512], BF16, tag="pt", name=f"pt{tag}")
            for u in range(2):
                for oi in range(2):
                    nc.tensor.transpose(pt[:, u * 256 + oi * P:
                                           u * 256 + (oi + 1) * P],
                                        nonT[:, oi, ts(2 * cg + u, P)],
                                        identb[:])
            _cp(nc, ci, weff[:, 2 * cg:2 * cg + 2, o_off:o_off + 256],
                pt[:].rearrange("p (u f) -> p u f", u=2))
            ci += 1


def _build():
    nc = bacc.Bacc("TRN2", target_bir_lowering=False, debug=False,
                   enable_asserts=False, num_devices=NCORES)

    d_xt = nc.dram_tensor("xt", (C, NT), BF16, kind="ExternalInput").ap()
    d_wqd = nc.dram_tensor("wqd", (Q_RANK, C), BF16, kind="ExternalInput").ap()
    d_wqu = nc.dram_tensor("wqu", (Q_RANK, 2 * P), BF16, kind="ExternalInput").ap()
    d_wkd = nc.dram_tensor("wkd", (KV_RANK, C), BF16, kind="ExternalInput").ap()
    d_wku = nc.dram_tensor("wku", (KV_RANK, 4 * P), BF16, kind="ExternalInput").ap()
    d_wot = nc.dram_tensor("wot", (HPC * V_DIM, C), BF16, kind="ExternalInput").ap()
    d_cos = nc.dram_tensor("cos128", (P, NT), BF16, kind="ExternalInput").ap()
    d_sin = nc.dram_tensor("sin128", (P, NT), BF16, kind="ExternalInput").ap()
    d_masks = nc.dram_tensor("masks", (P, 4 * TCH), BF16, kind="ExternalInput").ap()
    d_ones8 = nc.dram_tensor("ones8", (P, 8), BF16, kind="ExternalInput").ap()
    d_out = nc.dram_tensor("out", (NT, C), F32, kind="ExternalOutput").ap()

    xt_r = d_xt.rearrange("(kt p) t -> p kt t", p=P)

    with tile.TileContext(nc, pool_alloc_mode="queue") as tc:
        with tc.tile_pool(name="small", bufs=1) as sp, \
             tc.tile_pool(name="dram", bufs=1, space="DRAM") as dp:
            identb = sp.tile([P, P], BF16, tag="identb", name="identb")
            make_identity(nc, identb[:])
            ones8 = sp.tile([P, 8], BF16, tag="ones8", name="ones8")
            nc.sync.dma_start(ones8[:], d_ones8)
            masks = sp.tile([P, 4, TCH], BF16, tag="masks", name="masks")
            nc.sync.dma_start(masks[:], d_masks.rearrange("p (m w) -> p m w", w=TCH))
            epsl = sp.tile([P, 1], F32, tag="epsl", name="epsl")
            nc.gpsimd.memset(epsl[:], EPS128)
            epss = sp.tile([P, 1], F32, tag="epss", name="epss")
            nc.gpsimd.memset(epss[:], EPS)
            wot_t = sp.tile([P, HPC, C], BF16, tag="wot", name="wot")

            # persistent activation storage
            qattn = [sp.tile([P, NT], BF16, tag=f"qattn{h}", name=f"qattn{h}")
                     for h in range(HPC)]
            kattn = [sp.tile([P, NT], BF16, tag=f"kattn{h}", name=f"kattn{h}")
                     for h in range(HPC)]
            vtm = [sp.tile([P, 2 * TQT, V_DIM], BF16, tag=f"vtm{h}",
                           name=f"vtm{h}") for h in range(HPC)]
            rk = [sp.tile([P, 2 * TQT], F32, tag=f"rk{h}", name=f"rk{h}")
                  for h in range(HPC)]
            yt = [sp.tile([P, T], BF16, tag=f"yt{h}", name=f"yt{h}")
                  for h in range(HPC)]

            # ---------- folds (both down matrices SBUF-resident) ----------
            with tc.tile_pool(name="foldw", bufs=1) as fw:
                down_k = fw.tile([P, KK, C], BF16, tag="down_k", name="down_k")
                nc.sync.dma_start(down_k[:], d_wkd.rearrange("(kt p) c -> p kt c", p=P))
                up_k = fw.tile([P, KK, 4 * P], BF16, tag="up_k", name="up_k")
                nc.sync.dma_start(up_k[:], d_wku.rearrange("(kt p) m -> p kt m", p=P))
                down_q = fw.tile([P, KQ, C], BF16, tag="down_q", name="down_q")
                nc.sync.dma_start(down_q[:], d_wqd.rearrange("(kt p) c -> p kt c", p=P))
                up_q = fw.tile([P, KQ, 2 * P], BF16, tag="up_q", name="up_q")
                nc.sync.dma_start(up_q[:], d_wqu.rearrange("(kt p) m -> p kt m", p=P))
                nc.sync.dma_start(wot_t[:], d_wot.rearrange("(h p) c -> p h c", p=P))

                weff_q = sp.tile([P, KC, 2 * P], BF16, tag="weff_q", name="weff_q")
                weff_kv = sp.tile([P, KC, 4 * P], BF16, tag="weff_kv", name="weff_kv")
                nonT = fw.tile([P, 2, C], BF16, tag="nonT", name="nonT", bufs=2)

                _fold(nc, tc, "k0", down_k, up_k, KK, (0, 1), 0, weff_kv, nonT, identb)
                _fold(nc, tc, "k1", down_k, up_k, KK, (2, 3), 256, weff_kv, nonT, identb)
                _fold(nc, tc, "q", down_q, up_q, KQ, (0, 1), 0, weff_q, nonT, identb)

            cos_t = sp.tile([P, NT], BF16, tag="cosT", name="cosT")
            sin_t = sp.tile([P, NT], BF16, tag="sinT", name="sinT")
            nc.sync.dma_start(cos_t[:], d_cos)
            nc.sync.dma_start(sin_t[:], d_sin)

            # ---------- fused apply loop ----------
            with tc.tile_pool(name="apx", bufs=1) as xp, \
                 tc.tile_pool(name="aps", bufs=1) as asb, \
                 tc.tile_pool(name="qkps", bufs=1, space="PSUM") as qkp, \
                 tc.tile_pool(name="vps", bufs=1, space="PSUM") as vpp, \
                 tc.tile_pool(name="nps", bufs=1, space="PSUM") as npp:
                for i in range(NCH):
                    ci = i  # copy-engine alternation seed
                    xc = xp.tile([P, KC, TCH], BF16, tag="xc", name="xc", bufs=3)
                    nc.gpsimd.dma_start(xc[:], xt_r[:, :, ts(i, TCH)])
                    X1 = asb.tile([P, TCH], BF16, tag="X1", name="X1", bufs=2)
                    X2 = asb.tile([P, TCH], BF16, tag="X2", name="X2", bufs=2)
                    # q/k feature-major
                    for kind, wsrc, o in (("qn", weff_q, 0), ("qr", weff_q, 1),
                                          ("kn", weff_kv, 0), ("kr", weff_kv, 1)):
                        ps = qkp.tile([P, TCH], F32, tag=f"a{kind}", name=f"a{kind}")
                        for k in range(KC):
                            nc.tensor.matmul(ps[:], wsrc[:, k, ts(o, P)],
                                             xc[:, k, :],
                                             start=(k == 0), stop=(k == KC - 1))
                        if kind == "qn":
                            _cp(nc, ci, qattn[0][0:64, ts(i, TCH)], ps[0:64, :])
                            _cp(nc, ci + 1, qattn[1][0:64, ts(i, TCH)], ps[64:128, :])
                        elif kind == "qr":
                            _cp(nc, ci, X1[0:64, :], ps[0:64, :])
                            _cp(nc, ci + 1, X2[0:64, :], ps[64:128, :])
                        elif kind == "kn":
                            _cp(nc, ci, kattn[0][0:64, ts(i, TCH)], ps[0:64, :])
                            _cp(nc, ci + 1, kattn[1][0:64, ts(i, TCH)], ps[64:128, :])
                        else:
                            _cp(nc, ci, X1[64:128, :], ps[0:64, :])
                            _cp(nc, ci + 1, X2[64:128, :], ps[64:128, :])
                        ci += 2
                    # v token-major
                    for half in range(2):
                        ps = vpp.tile([P, 2, 2 * V_DIM], F32, tag="vp", name="vp",
                                      bufs=2)
                        for lo in range(2):
                            tt = half * 2 + lo
                            for k in range(KC):
                                nc.tensor.matmul(ps[:, lo, :],
                                                 xc[:, k, ts(tt, P)],
                                                 weff_kv[:, k, 256:512],
                                                 start=(k == 0), stop=(k == KC - 1))
                        for lo in range(2):
                            g = i * 4 + half * 2 + lo
                            _cp(nc, ci, vtm[0][:, g, :], ps[:, lo, 0:V_DIM])
                            _cp(nc, ci + 1, vtm[1][:, g, :], ps[:, lo, V_DIM:])
                            ci += 2
                    # RoPE on the packed 128-row stack
                    lo_t = asb.tile([P, TCH], BF16, tag="lo", name="lo", bufs=2)
                    hi_t = asb.tile([P, TCH], BF16, tag="hi", name="hi", bufs=2)
                    tm1 = asb.tile([P, TCH], BF16, tag="tm1", name="tm1", bufs=2)
                    cosc = cos_t[:, ts(i, TCH)]
                    sinc = sin_t[:, ts(i, TCH)]
                    nc.vector.tensor_tensor(lo_t[:], X1[:], cosc, MULT)
                    nc.vector.tensor_tensor(tm1[:], X2[:], sinc, MULT)
                    nc.vector.tensor_add(lo_t[:], lo_t[:], tm1[:])
                    nc.vector.tensor_tensor(hi_t[:], X2[:], cosc, MULT)
                    nc.vector.tensor_tensor(tm1[:], X1[:], sinc, MULT)
                    nc.vector.tensor_sub(hi_t[:], hi_t[:], tm1[:])
                    for qi, att in ((0, qattn), (1, kattn)):
                        for h in range(HPC):
                            r0 = qi * 64 + h * 32
                            nc.sync.dma_start(att[h][64:96, ts(i, TCH)],
                                              lo_t[r0:r0 + 32, :])
                            nc.sync.dma_start(att[h][96:128, ts(i, TCH)],
                                              hi_t[r0:r0 + 32, :])
                    # RMS-norm scales
                    col0 = 16 * (i // 4) + 4 * (i % 4)
                    for h in range(HPC):
                        # q: rq = 1/sqrt(sumsq + 128 eps); multiply into qattn
                        sq = asb.tile([P, TCH], BF16, tag="sq", name="sq", bufs=2)
                        nc.scalar.activation(sq[:], qattn[h][:, ts(i, TCH)], SQUARE)
                        nps = npp.tile([P, TCH], F32, tag="nrm", name="nq", bufs=2)
                        nc.tensor.matmul(nps[0:8, :], ones8[:], sq[:])
                        rts = asb.tile([8, TCH], F32, tag="rts", name="rts", bufs=2)
                        nc.scalar.activation(rts[:], nps[0:8, :], SQRT,
                                             bias=epsl[0:8])
                        rqs = asb.tile([8, TCH], F32, tag="rqs", name="rqs", bufs=2)
                        nc.vector.reciprocal(rqs[:], rts[:])
                        scq = dp.tile([1, TCH], F32, tag=f"scq{i}_{h}",
                                      name=f"scq{i}_{h}")
                        nc.sync.dma_start(scq[:], rqs[0:1, :])
                        rqb = asb.tile([P, TCH], F32, tag="rqb", name="rqb", bufs=2)
                        nc.sync.dma_start(rqb[:], scq[0:1, :].to_broadcast((P, TCH)))
                        nc.vector.tensor_tensor(qattn[h][:, ts(i, TCH)],
                                                qattn[h][:, ts(i, TCH)],
                                                rqb[:], MULT)
                        # k: rk = 1/rms = 1/sqrt(sumsq/128 + eps), kept per-token
                        sqk = asb.tile([P, TCH], BF16, tag="sqk", name="sqk", bufs=2)
                        nc.scalar.activation(sqk[:], kattn[h][:, ts(i, TCH)], SQUARE)
                        npk = npp.tile([P, TCH], F32, tag="nrm", name="nk", bufs=2)
                        nc.tensor.matmul(npk[0:8, :], ones8[:], sqk[:])
                        rtk = asb.tile([8, TCH], F32, tag="rtk", name="rtk", bufs=2)
                        nc.scalar.activation(rtk[:], npk[0:8, :], SQRT,
                                             scale=1.0 / HEAD_DIM, bias=epss[0:8])
                        rks = asb.tile([8, TCH], F32, tag="rks", name="rks", bufs=2)
                        nc.vector.reciprocal(rks[:], rtk[:])
                        sck = dp.tile([1, TCH], F32, tag=f"sck{i}_{h}",
                                      name=f"sck{i}_{h}")
                        nc.sync.dma_start(sck[:], rks[0:1, :])
                        nc.sync.dma_start(rk[h][:, col0:col0 + 4],
                                          sck[:].rearrange("o (g p) -> o p g", p=P)[0])

            # ---------- attention (S transposed) + wo ----------
            with tc.tile_pool(name="attw", bufs=1) as aw, \
                 tc.tile_pool(name="sps", bufs=1, space="PSUM") as sps, \
                 tc.tile_pool(name="yps", bufs=1, space="PSUM") as yps, \
                 tc.tile_pool(name="dnps", bufs=1, space="PSUM") as dnp, \
                 tc.tile_pool(name="wops", bufs=1, space="PSUM") as wps:
                for b in range(B):
                    for h in range(HPC):
                        q_bh = qattn[h][:, ts(b, T)]
                        k_bh = kattn[h][:, ts(b, T)]
                        for cg in range(2):
                            dn = dnp.tile([P, TCH], F32, tag="dn", name="dn",
                                          bufs=2)
                            ys = [yps.tile([P, TCH], F32, tag=f"y{cl}",
                                           name=f"y{cl}", bufs=1)
                                  for cl in range(2)]
                            for j in range(8 * cg + 8):
                                pxs = {}
                                for cl in range(2):
                                    c = 2 * cg + cl
                                    if j > 4 * c + 3:
                                        continue
                                    spt = sps.tile([P, TCH], F32, tag="sp",
                                                   name="sp", bufs=2)
                                    nc.tensor.matmul(spt[:], k_bh[:, ts(j, P)],
                                                     q_bh[:, ts(c, TCH)])
                                    px = aw.tile([P, TCH], BF16, tag="px",
                                                 name="px", bufs=6)
                                    nc.scalar.activation(
                                        px[:], spt[:], EXP,
                                        scale=rk[h][:, 16 * b + j:16 * b + j + 1])
                                    if j // 4 == c:
                                        nc.vector.tensor_tensor(
                                            px[:], px[:], masks[:, j % 4, :], MULT)
                                    pxs[cl] = px
                                for cl, px in pxs.items():
                                    c = 2 * cg + cl
                                    nc.tensor.matmul(dn[32 * cl:32 * cl + 8, :],
                                                     ones8[:], px[:],
                                                     start=(j == 0),
                                                     stop=(j == 4 * c + 3))
                                for cl, px in pxs.items():
                                    c = 2 * cg + cl
                                    nc.tensor.matmul(ys[cl][:],
                                                     vtm[h][:, 16 * b + j, :],
                                                     px[:],
                                                     start=(j == 0),
                                                     stop=(j == 4 * c + 3))
                            for cl in range(2):
                                c = 2 * cg + cl
                                rdv = aw.tile([1, TCH], F32, tag="rdv",
                                              name="rdv", bufs=2)
                                nc.vector.reciprocal(rdv[:],
                                                     dn[32 * cl:32 * cl + 1, :])
                                scd = dp.tile([1, TCH], F32,
                                              tag=f"scd{b}_{h}_{c}",
                                              name=f"scd{b}_{h}_{c}")
                                nc.sync.dma_start(scd[:], rdv[:])
                                rdb = aw.tile([P, TCH], F32, tag="rdb",
                                              name="rdb", bufs=2)
                                nc.sync.dma_start(rdb[:],
                                                  scd[0:1, :].to_broadcast((P, TCH)))
                                nc.vector.tensor_tensor(yt[h][:, ts(c, TCH)],
                                                        ys[cl][:], rdb[:], MULT)
                                if h == HPC - 1:
                                    for mi in range(4 * c, 4 * c + 4):
                                        stg = aw.tile([P, C], BF16, tag="stg",
                                                      name="stg", bufs=2)
                                        for npair in range(2):
                                            wp = wps.tile([P, 2, 512], F32,
                                                          tag="wo", name="wo",
                                                          bufs=1)
                                            for h2 in range(HPC):
                                                for nl in range(2):
                                                    n = 2 * npair + nl
                                                    nc.tensor.matmul(
                                                        wp[:, nl, :],
                                                        yt[h2][:, ts(mi, P)],
                                                        wot_t[:, h2, ts(n, 512)],
                                                        start=(h2 == 0),
                                                        stop=(h2 == HPC - 1))
                                            _cp(nc, mi + npair,
                                                stg[:, ts(npair, 1024)],
                                                wp[:].rearrange("p a b -> p (a b)"))
                                        nc.gpsimd.dma_start(
                                            d_out[ts(16 * b + mi, P), :], stg[:])

    nc.compile()
    return nc


def _host_prep(x, cos, sin, wq_down, wq_up, wkv_down, wkv_up, wo):
    import ml_dtypes
    bf16 = ml_dtypes.bfloat16
    x_t = np.ascontiguousarray(
        np.asarray(x, dtype=np.float32).reshape(NT, C).T.astype(bf16))  # [C, NT]
    cos_t = np.asarray(cos, dtype=np.float32)[0, :, 0, :].T        # [32, T]
    sin_t = np.asarray(sin, dtype=np.float32)[0, :, 0, :].T
    cos128 = np.ascontiguousarray(
        np.tile(np.tile(cos_t, (4, 1)), (1, B)).astype(bf16))
    sin128 = np.ascontiguousarray(
        np.tile(np.tile(sin_t, (4, 1)), (1, B)).astype(bf16))

    # causal masks for the 4 diagonal positions of a 512-wide q chunk
    masks = np.zeros((P, 4 * TCH), dtype=np.float32)
    triu = np.triu(np.ones((P, P), dtype=np.float32))
    for jm in range(4):
        m = np.zeros((P, TCH), dtype=np.float32)
        m[:, jm * P:(jm + 1) * P] = triu
        m[:, (jm + 1) * P:] = 1.0
        masks[:, jm * TCH:(jm + 1) * TCH] = m
    masks = np.ascontiguousarray(masks.astype(bf16))
    ones8 = np.ones((P, 8), dtype=np.float32).astype(bf16)

    wq_up = np.asarray(wq_up, dtype=np.float32)
    wkv_up = np.asarray(wkv_up, dtype=np.float32)
    wo = np.asarray(wo, dtype=np.float32)
    wq_down = np.ascontiguousarray(
        np.asarray(wq_down, dtype=np.float32).astype(bf16))
    wkv_down = np.ascontiguousarray(
        np.asarray(wkv_down, dtype=np.float32).astype(bf16))

    in_maps = []
    for core in range(NCORES):
        h0, h1 = HPC * core, HPC * core + 1
        qrows = ([h0 * HEAD_DIM + d for d in range(64)]
                 + [h1 * HEAD_DIM + d for d in range(64)]
                 + [h0 * HEAD_DIM + 64 + d for d in range(32)]
                 + [h1 * HEAD_DIM + 64 + d for d in range(32)]
                 + [h0 * HEAD_DIM + 96 + d for d in range(32)]
                 + [h1 * HEAD_DIM + 96 + d for d in range(32)])
        KVD = HEAD_DIM + V_DIM
        krows = ([h0 * KVD + d for d in range(64)]
                 + [h1 * KVD + d for d in range(64)]
                 + [h0 * KVD + 64 + d for d in range(32)]
                 + [h1 * KVD + 64 + d for d in range(32)]
                 + [h0 * KVD + 96 + d for d in range(32)]
                 + [h1 * KVD + 96 + d for d in range(32)]
                 + [h0 * KVD + HEAD_DIM + d for d in range(V_DIM)]
                 + [h1 * KVD + HEAD_DIM + d for d in range(V_DIM)])
        wqu_slice = np.ascontiguousarray(wq_up[qrows, :].T.astype(bf16))
        wku_slice = np.ascontiguousarray(wkv_up[krows, :].T.astype(bf16))
        ocols = ([h0 * V_DIM + d for d in range(V_DIM)]
                 + [h1 * V_DIM + d for d in range(V_DIM)])
        wot_slice = np.ascontiguousarray(wo[:, ocols].T.astype(bf16))
        in_maps.append({
            "xt": x_t, "wqd": wq_down, "wqu": wqu_slice,
            "wkd": wkv_down, "wku": wku_slice, "wot": wot_slice,
            "cos128": cos128, "sin128": sin128, "masks": masks, "ones8": ones8,
        })
    return in_maps


def kernel(x, cos, sin, wq_down, wq_up, wkv_down, wkv_up, wo):
    global _last_results
    if "nc" not in _CACHE:
        _CACHE["nc"] = _build()
    nc = _CACHE["nc"]
    in_maps = _host_prep(x, cos, sin, wq_down, wq_up, wkv_down, wkv_up, wo)
    res = run_bass_kernel_spmd(nc, in_maps, core_ids=list(range(NCORES)))
    _last_results = res
    acc = res.results[0]["out"].astype(np.float32)
    for corer in res.results[1:]:
        acc = acc + corer["out"]
    return acc.reshape(B, T, C)
